# revision 4
# baseline (speedup 1.0000x reference)
"""Trainium2 Bass kernel for nn_GCNStacking: 3-layer dense-adjacency GraphConv.

Per batch element b (one per NeuronCore, B=8 = n_cores=8, pure data parallel):
    H = relu(A @ (X @ Wm0^T) + X @ Ws0^T + b0)
    H = relu(A @ (H @ Wm1^T) + H @ Ws1^T + b1)
    H =      A @ (H @ Wm2^T) + H @ Ws2^T + b2

Dataflow (per core), everything kept in "transposed state" Ht = H^T [C=64, N=2048]:
  - A^T is materialized once in SBUF (f32r) via pipelined PE transposes while
    the 16 row-slabs of A stream in from HBM; reused by all 3 layers.
  - Message M_l (natural [N, C] layout) via 16 small matmuls lhsT=Ht-block.
  - Aggregation Ot = (A @ M)^T via matmuls lhsT=M-block [128,64] (stationary),
    rhs=A^T-block [128,512] (moving, f32r full-rate), col-packed 2-wide into
    the 128-wide PE array (even j-blocks -> psum partitions 0:64, odd -> 64:128).
  - Self term Ws @ Ht accumulated into the same PSUM; bias+relu on DVE evac.
  - Final layer transposed back to natural [N, C] via PE transposes and DMA'd out.

float32r is bit-identical storage to f32 but matmuls run at full PE rate
(fp32 runs 1/4 rate); operands are rounded to ~12 mantissa bits by the DVE
copies that produce them (measured 1.5e-4 elementwise; ~1e-3 end-to-end here).
Set AGG_F32R=False to run everything in exact fp32 (about 2x slower).
"""
import sys

for _p in ("/opt/trn_rl_repo",):
    if _p not in sys.path:
        sys.path.insert(0, _p)

import numpy as np
import orjson

import concourse.bass as bass
import concourse.tile as tile
from concourse import mybir
from concourse.bass import _add_dep_helper as add_dep

f32 = mybir.dt.float32
f32r = mybir.dt.float32r

AGG_F32R = True

# ---------------------------------------------------------------------------
# Workaround: this walrus build accepts at most ONE embedded sync-wait per
# instruction ("Too many sync wait commands").  Split excess waits onto
# inserted NoOps (same engine, right before the host instruction).
# ---------------------------------------------------------------------------
_ws_ctr = [0]


def _split_waits_json(bir_bytes: bytes) -> bytes:
    d = orjson.loads(bir_bytes)
    changed = False
    for fn in d.get("functions", []):
        for blk in fn.get("blocks", []):
            out = []
            for inst in blk.get("instructions", []):
                si = inst.get("sync_info")
                waits = (si or {}).get("on_wait") or []
                eng = inst.get("engine")
                if len(waits) > 1 and eng and eng != "Unassigned":
                    changed = True
                    for w in waits[:-1]:
                        _ws_ctr[0] += 1
                        out.append({
                            "name": f"I-wsplit-{_ws_ctr[0]}",
                            "opcode": "NoOp",
                            "engine": eng,
                            "ins": [],
                            "outs": [],
                            "sync_info": {"on_wait": [w], "on_update": []},
                        })
                    si["on_wait"] = waits[-1:]
                out.append(inst)
            blk["instructions"] = out
    return orjson.dumps(d) if changed else bir_bytes


def _install_waitsplit():
    from concourse import bass2jax, bass_utils
    if getattr(bass_utils, "_waitsplit_installed", False):
        return
    orig = bass_utils.compile_bir_kernel

    def patched(bir_json, tmpdir, neff_name="file.neff"):
        return orig(_split_waits_json(bytes(bir_json)), tmpdir, neff_name=neff_name)

    bass_utils.compile_bir_kernel = patched
    bass2jax.compile_bir_kernel = patched
    bass_utils._waitsplit_installed = True


_install_waitsplit()

# ---------------------------------------------------------------------------
# Kernel builder
# ---------------------------------------------------------------------------
P = 128
C = 64
N_LAYERS = 3


def build_gcn(nn_nodes: int = 2048):
    """Build the single-core Bass program; the same program runs SPMD on all
    8 cores with per-core (per-batch) inputs."""
    NN = nn_nodes
    NB = NN // P            # node blocks (16)
    CH = 512                # aggregation i-chunk (one PSUM bank of f32)
    IC = NN // CH           # i-chunks (4)
    SLAB_PAIRS = 2          # A-slabs transposed per psum tile

    rdt = f32r if AGG_F32R else f32

    nc = bass.Bass()
    X_in = nc.declare_dram_parameter("X", [NN, C], f32, isOutput=False)
    A_in = nc.declare_dram_parameter("A", [NN, NN], f32, isOutput=False)
    W_in = {}
    b_in = {}
    for l in range(N_LAYERS):
        W_in[(l, "m")] = nc.declare_dram_parameter(f"Wm{l}", [C, C], f32, isOutput=False)
        W_in[(l, "s")] = nc.declare_dram_parameter(f"Ws{l}", [C, C], f32, isOutput=False)
        b_in[l] = nc.declare_dram_parameter(f"b{l}", [C], f32, isOutput=False)
    H_out = nc.declare_dram_parameter("H", [NN, C], f32, isOutput=True)

    with tile.TileContext(nc) as tc:
        with (
            tc.tile_pool(name="const", bufs=1) as const,
            tc.tile_pool(name="ht_pool", bufs=2) as ht_pool,
            tc.tile_pool(name="mn_pool", bufs=2) as mn_pool,
            tc.tile_pool(name="slab_pool", bufs=3) as slab_pool,
            tc.tile_pool(name="u_pool", bufs=2) as u_pool,
            tc.tile_pool(name="hb_pool", bufs=4) as hb_pool,
            tc.tile_pool(name="ps_tr", bufs=2, space="PSUM") as ps_tr,
            tc.tile_pool(name="ps_o", bufs=2, space="PSUM") as ps_o,
            tc.tile_pool(name="ps_m", bufs=2, space="PSUM") as ps_m,
        ):
            # ---- phase 0: constants, X^T, W^T -----------------------------
            ident = const.tile([P, P], f32, name="ident")
            id_i1 = nc.gpsimd.memset(ident, 0.0)
            id_i2 = nc.gpsimd.affine_select(
                out=ident, in_=ident,
                compare_op=mybir.AluOpType.not_equal,
                fill=1.0, base=0, pattern=[[-1, P]], channel_multiplier=1,
            )

            x_sb = const.tile([P, NB, C], f32, name="x_sb")
            x_dma = nc.sync.dma_start(
                x_sb, X_in[:].rearrange("(nb p) c -> p nb c", p=P))

            w_stage = {}
            w_dmas = []
            for l in range(N_LAYERS):
                for kind in ("m", "s"):
                    wst = const.tile([C, C], f32, name=f"wst_{l}{kind}")
                    w_dmas.append(nc.sync.dma_start(wst, W_in[(l, kind)][:]))
                    w_stage[(l, kind)] = wst
            b_sb = {}
            for l in range(N_LAYERS):
                bt = const.tile([C, 1], f32, name=f"b_sb{l}")
                nc.sync.dma_start(bt, b_in[l][:].rearrange("(p o) -> p o", o=1))
                b_sb[l] = bt

            # gate: one PE nop absorbing phase-0 input waits so the f32
            # transposes below carry at most one embedded wait each
            gate0 = nc.tensor.nop(nofuse=True)
            for d in (id_i1, id_i2, x_dma, *w_dmas):
                add_dep(gate0.ins, d.ins, True, "phase0 gate")

            # Ht[l]: transposed state [C, NN]; Ht[0] = X^T
            Ht = [ht_pool.tile([C, NN], rdt, name=f"Ht{l}", tag="ht")
                  for l in range(N_LAYERS)]
            for nb in range(NB):
                pt = ps_tr.tile([P, CH], f32, name="pt_x", tag="tr")
                t = nc.tensor.transpose(pt[:C, :P], x_sb[:, nb, :], ident)
                add_dep(t.ins, gate0.ins, False, "after gate0")
                nc.vector.tensor_copy(Ht[0][:, nb * P:(nb + 1) * P], pt[:C, :P])

            wT = {}
            for (l, kind), wst in w_stage.items():
                pw = ps_tr.tile([P, CH], f32, name="pt_w", tag="tr")
                t = nc.tensor.transpose(pw[:C, :C], wst, ident[:C, :C])
                add_dep(t.ins, gate0.ins, False, "after gate0")
                wt = const.tile([C, C], rdt, name=f"wT_{l}{kind}")
                nc.vector.tensor_copy(wt, pw[:C, :C])
                wT[(l, kind)] = wt

            # resident A^T, f32r, [j-partition, j-block, i]
            ATr = const.tile([P, NB, NN], rdt, name="ATr")

            def emit_mprod(l, mn):
                """M_l natural [N, C] blocks: lhsT = Ht[l] block, rhs = WmT."""
                for jb in range(NB):
                    pm = ps_m.tile([P, C], f32, name="pm", tag="m")
                    nc.tensor.matmul(pm, Ht[l][:, jb * P:(jb + 1) * P],
                                     wT[(l, "m")], start=True, stop=True)
                    nc.vector.tensor_copy(mn[:, jb, :], pm)

            def emit_agg_chunk(l, g, mn):
                """One i-chunk of Ot = (A@M + H@Ws^T)^T.

                 4-byte matmuls may only target PSUM partition base 0
                (s3d3_mm_valid_dst_partition), so no column-packing: one
                16-deep accumulation chain over j-blocks plus the self term.
                """
                po = ps_o.tile([C, CH], f32, name="po", tag="o")
                for jb in range(NB):
                    nc.tensor.matmul(
                        po, mn[:, jb, :],
                        ATr[:, jb, g * CH:(g + 1) * CH],
                        start=(jb == 0), stop=False,
                        skip_group_check=True,
                    )
                nc.tensor.matmul(
                    po, wT[(l, "s")], Ht[l][:, g * CH:(g + 1) * CH],
                    start=False, stop=True, skip_group_check=True,
                )
                if l < N_LAYERS - 1:
                    # Ht[l+1] chunk = relu(po + b_l)
                    nc.vector.tensor_scalar(
                        Ht[l + 1][:, g * CH:(g + 1) * CH], po,
                        b_sb[l], 0.0,
                        mybir.AluOpType.add, mybir.AluOpType.max)
                else:
                    ho = u_pool.tile([C, CH], f32, name="ho", tag="ho")
                    nc.vector.tensor_scalar(ho, po, b_sb[l], None,
                                            mybir.AluOpType.add)
                    for k in range(CH // P):
                        ph = ps_tr.tile([P, CH], f32, name="ph", tag="tr")
                        nc.tensor.transpose(ph[:, :C], ho[:, k * P:(k + 1) * P],
                                            ident[:C, :C])
                        hb = hb_pool.tile([P, C], f32, name="hb", tag="hb")
                        nc.vector.tensor_copy(hb, ph[:, :C])
                        r0 = g * CH + k * P
                        nc.sync.dma_start(H_out[r0:r0 + P, :], hb)

            # ---- layer 1, pipelined with the A load/transpose -------------
            mn1 = mn_pool.tile([P, NB, C], rdt, name="mn", tag="mn")
            emit_mprod(0, mn1)

            for g in range(IC):
                # 4 slabs of A rows -> transposed into ATr columns for chunk g
                for pair in range(CH // (P * SLAB_PAIRS)):
                    slabs = []
                    for si in range(SLAB_PAIRS):
                        s = g * (CH // P) + pair * SLAB_PAIRS + si
                        a_sl = slab_pool.tile([P, NN], f32, name="a_slab",
                                              tag="aslab")
                        d = nc.sync.dma_start(a_sl, A_in[s * P:(s + 1) * P, :])
                        slabs.append((a_sl, d, s))
                    gate = nc.tensor.nop(nofuse=True)
                    for _, d, _s in slabs:
                        add_dep(gate.ins, d.ins, True, "slab gate")
                    for jb in range(NB):
                        pt = ps_tr.tile([P, CH], f32, name="pt_a", tag="tr")
                        for si, (a_sl, _d, _s) in enumerate(slabs):
                            t = nc.tensor.transpose(
                                pt[:, si * P:(si + 1) * P],
                                a_sl[:, jb * P:(jb + 1) * P], ident)
                            add_dep(t.ins, gate.ins, False, "after slab gate")
                        c0 = (g * CH + pair * SLAB_PAIRS * P)
                        nc.vector.tensor_copy(
                            ATr[:, jb, c0:c0 + SLAB_PAIRS * P],
                            pt[:, :SLAB_PAIRS * P])
                emit_agg_chunk(0, g, mn1)

            # ---- layers 2..3 ---------------------------------------------
            for l in range(1, N_LAYERS):
                mn = mn_pool.tile([P, NB, C], rdt, name="mn", tag="mn")
                emit_mprod(l, mn)
                for g in range(IC):
                    emit_agg_chunk(l, g, mn)

    return nc


# ---------------------------------------------------------------------------
# Harness entry point
# ---------------------------------------------------------------------------
_NC_CACHE = {}


def _get_nc(nn_nodes):
    if nn_nodes not in _NC_CACHE:
        _NC_CACHE[nn_nodes] = build_gcn(nn_nodes)
    return _NC_CACHE[nn_nodes]


def kernel(X, A, Wm0, Ws0, b0, Wm1, Ws1, b1, Wm2, Ws2, b2, _trace=False):
    from concourse.bass_utils import run_bass_kernel_spmd

    X = np.ascontiguousarray(np.asarray(X, dtype=np.float32))
    A = np.ascontiguousarray(np.asarray(A, dtype=np.float32))
    B, NN, _C = X.shape
    assert B == 8, f"expected batch 8 (one per core), got {B}"

    shared = {
        "Wm0": np.ascontiguousarray(np.asarray(Wm0, np.float32)),
        "Ws0": np.ascontiguousarray(np.asarray(Ws0, np.float32)),
        "b0": np.ascontiguousarray(np.asarray(b0, np.float32)),
        "Wm1": np.ascontiguousarray(np.asarray(Wm1, np.float32)),
        "Ws1": np.ascontiguousarray(np.asarray(Ws1, np.float32)),
        "b1": np.ascontiguousarray(np.asarray(b1, np.float32)),
        "Wm2": np.ascontiguousarray(np.asarray(Wm2, np.float32)),
        "Ws2": np.ascontiguousarray(np.asarray(Ws2, np.float32)),
        "b2": np.ascontiguousarray(np.asarray(b2, np.float32)),
    }
    nc = _get_nc(NN)
    in_maps = [dict(shared, X=X[b], A=A[b]) for b in range(B)]
    res = run_bass_kernel_spmd(nc, in_maps, core_ids=list(range(B)),
                               trace=_trace)
    out = np.stack([res.results[b]["H"] for b in range(B)], axis=0)
    if _trace:
        return out, res
    return out
